# revision 1
# baseline (speedup 1.0000x reference)
"""Trainium2 Bass kernel for nn_CapsuleNetwork (capsule routing, bilinear_type=2).

Precision note: the routing logits |cw| reach ~230 with top-2 gaps as small
as 0.1, so the softmax acts as a near-argmax: any 16-bit rounding of the
bilinear (hat) flips winners and produces O(1) output errors.  Everything is
therefore computed in fp32.

Strategy (pure data parallel over batch, 8 cores x 128 samples):
  - Host prep (layout only): x -> xT[s, h, b] f32, w -> wT[s, h, i*h] f32,
    mask -> f32.
  - Per core, 4 sequential phases (one per capsule i; routing is
    i-independent).  Per phase: PE computes hat_i[s] = xT_s^T @ wT_s[:,i]
    ([128b, 128h] f32, 4 cyc/row), evacuated to SBUF f32 (102.4KB/part,
    coexists with resident xT f32).  Routing for that i runs on-chip:
    cap accumulation via scalar_tensor_tensor FMA, delta via chunked
    tensor_mul + tensor_reduce, softmax via reduce_max(negate) +
    Exp(accum_out) + reciprocal.  Small state lives in PSUM.
"""

import os
import sys

for _p in ("/opt/trn_rl_repo", "/root/.axon_site/_ro/trn_rl_repo"):
    if os.path.isdir(_p) and _p not in sys.path:
        sys.path.insert(0, _p)

from contextlib import ExitStack

import numpy as np

import concourse.bass as bass
import concourse.mybir as mybir
import concourse.tile as tile
from concourse.bass_utils import run_bass_kernel_spmd

B, S, I, H = 1024, 200, 4, 128
IH = I * H
NCORES = 8
BC = B // NCORES  # samples per core
EPS = 1e-9
CH = 2  # s-chunk for delta
F32 = mybir.dt.float32
ALU = mybir.AluOpType
ACTF = mybir.ActivationFunctionType

_cache = {}


def _legalize_waits(nc):
    """neuronxcc walrus codegen supports one sync-wait slot per TPB
    instruction; Tile emits several.  Split: for each instruction with k>1
    waits, prepend k-1 single-wait NoOps on the same engine (semantically
    identical — the engine blocks on each in turn)."""
    import bass_rust

    uid = [0]
    for func in nc.m.functions:
        for bb in func.blocks:
            insts = bb.instructions  # live view
            out = []
            changed = False
            for ins in insts:
                si = ins.sync_info
                waits = list(si.on_wait) if si is not None else []
                if len(waits) > 1:
                    changed = True
                    for w in waits[:-1]:
                        nop = mybir.InstNoOp(
                            name=f"wsplit-{uid[0]}", ins=[], outs=[]
                        )
                        uid[0] += 1
                        nop.engine = ins.engine
                        nop.sync_info = bass_rust.SyncInfo(
                            on_wait=[w], on_update=[]
                        )
                        out.append(nop)
                    si.on_wait = [waits[-1]]
                    ins.sync_info = si
                out.append(ins)
            if changed:
                insts.clear()
                insts.extend(out)


def _build_program():
    nc = bass.Bass(target_bir_lowering=False, trn_type="TRN2")
    xt_d = nc.declare_dram_parameter("xt", [S, H, BC], F32, isOutput=False)
    wt_d = nc.declare_dram_parameter("wt", [S, H, IH], F32, isOutput=False)
    msk_d = nc.declare_dram_parameter("msk", [BC, S], F32, isOutput=False)
    out_d = nc.declare_dram_parameter("out", [BC, IH], F32, isOutput=True)

    with ExitStack() as ctx:
        tc = ctx.enter_context(tile.TileContext(nc))

        big = ctx.enter_context(tc.tile_pool(name="big", bufs=1))
        small = ctx.enter_context(tc.tile_pool(name="small", bufs=1))

        xts = big.tile([H, S, BC], F32)  # resident xT, 102.4KB/part
        hat = big.tile([BC, S, H], F32)  # per-phase hat_i, 102.4KB/part
        msk_t = small.tile([BC, S], F32)
        cw1 = small.tile([BC, S], F32)
        capx = small.tile([BC, CH, H], F32)  # cap broadcast for delta chunks
        tmp0 = small.tile([BC, CH, H], F32)
        tmp1 = small.tile([BC, CH, H], F32)
        tmps = [tmp0, tmp1]
        junk = small.tile([BC, H], F32)
        epsc = small.tile([BC, 1], F32)
        stats = small.tile([BC, 12], F32)
        negmax1 = stats[:, 0:1]
        sumexp1 = stats[:, 1:2]
        recips1 = stats[:, 2:3]
        nrm1 = stats[:, 3:4]
        sq1 = stats[:, 4:5]
        np11 = stats[:, 5:6]
        den1 = stats[:, 6:7]
        rd1 = stats[:, 7:8]
        factor1 = stats[:, 8:9]

        nc.vector.memset(epsc, EPS)
        nc.sync.dma_start(out=msk_t, in_=msk_d[:, :])
        nc.sync.dma_start(
            out=xts, in_=xt_d[:, :, :].rearrange("s h b -> h s b")
        )

        for phase in range(I):
            jlo = phase * H

            # ---------- main bilinear for this i: hat[b, s, :] ----------
            with (
                tc.tile_pool(name=f"wp{phase}", bufs=3) as wp,
                tc.tile_pool(name=f"pm{phase}", bufs=3, space="PSUM") as pm,
            ):
                for s0 in range(0, S, 4):
                    ps4 = pm.tile([BC, 4, H], F32)
                    for c in range(4):
                        s = s0 + c
                        wt_t = wp.tile([H, H], F32)
                        nc.sync.dma_start(
                            out=wt_t, in_=wt_d[s, :, jlo : jlo + H]
                        )
                        nc.tensor.matmul(
                            ps4[:, c, :],
                            lhsT=xts[:, s, :],
                            rhs=wt_t,
                            start=True,
                            stop=True,
                        )
                    nc.scalar.activation(
                        out=hat[:, s0 : s0 + 4, :], in_=ps4, func=ACTF.Copy
                    )

            # ---------- routing for this i ----------
            with tc.tile_pool(name=f"pr{phase}", bufs=1, space="PSUM") as pr:
                cap = pr.tile([BC, H], F32)
                cw0 = pr.tile([BC, S], F32)
                sw = pr.tile([BC, S], F32)

                def softmax(cwx):
                    nc.vector.tensor_reduce(
                        out=negmax1,
                        in_=cwx,
                        axis=mybir.AxisListType.X,
                        op=ALU.max,
                        negate=True,
                    )
                    nc.scalar.activation(
                        out=sw,
                        in_=cwx,
                        func=ACTF.Exp,
                        bias=negmax1,
                        scale=1.0,
                        accum_out=sumexp1,
                    )
                    nc.vector.reciprocal(out=recips1, in_=sumexp1)
                    # sw = (exp * 1/sumexp) * mask   (in place)
                    nc.vector.scalar_tensor_tensor(
                        out=sw,
                        in0=sw,
                        scalar=recips1,
                        in1=msk_t,
                        op0=ALU.mult,
                        op1=ALU.mult,
                    )

                def squash(it):
                    nc.scalar.activation(
                        out=junk, in_=cap, func=ACTF.Square, accum_out=nrm1
                    )
                    if it == 0:
                        nc.vector.tensor_scalar_mul(nrm1, nrm1, 1.0 / (S * S))
                    nc.scalar.activation(
                        out=sq1, in_=nrm1, func=ACTF.Sqrt, bias=epsc, scale=1.0
                    )
                    nc.vector.tensor_scalar_add(np11, nrm1, 1.0)
                    nc.vector.tensor_mul(den1, np11, sq1)
                    nc.vector.reciprocal(out=rd1, in_=den1)
                    nc.vector.tensor_mul(factor1, nrm1, rd1)
                    if it == 0:
                        nc.vector.tensor_scalar_mul(factor1, factor1, 1.0 / S)
                    nc.vector.tensor_scalar_mul(cap, cap, factor1)

                for it in range(3):
                    if it > 0:
                        softmax(cw0 if it == 1 else cw1)

                    # cap = sum_s sw_s * hat_s (raw)
                    nc.vector.memset(cap, 0.0)
                    for s in range(S):
                        nc.vector.scalar_tensor_tensor(
                            out=cap,
                            in0=hat[:, s, :],
                            scalar=(
                                msk_t[:, s : s + 1]
                                if it == 0
                                else sw[:, s : s + 1]
                            ),
                            in1=cap,
                            op0=ALU.mult,
                            op1=ALU.add,
                        )

                    squash(it)

                    if it < 2:
                        # capx = cap broadcast over the s-chunk dim
                        for c in range(CH):
                            nc.vector.tensor_copy(out=capx[:, c, :], in_=cap)
                        cwx = cw0 if it == 0 else cw1
                        for k, s0 in enumerate(range(0, S, CH)):
                            # multiply on the otherwise-idle GpSimd engine
                            # (double-buffered tmp so it pipelines with the
                            # VectorE free-dim reduction); every third chunk's
                            # reduction goes to ScalarE (Copy + accum_out) to
                            # offload the bottleneck VectorE further
                            tmp = tmps[k % 2]
                            nc.gpsimd.tensor_mul(
                                tmp,
                                hat[:, s0 : s0 + CH, :],
                                capx,
                            )
                            if k % 3 == 0:
                                for c in range(CH):
                                    nc.scalar.activation(
                                        out=junk,
                                        in_=tmp[:, c, :],
                                        func=ACTF.Copy,
                                        accum_out=cwx[:, s0 + c : s0 + c + 1],
                                    )
                            else:
                                nc.vector.tensor_reduce(
                                    out=cwx[:, s0 : s0 + CH],
                                    in_=tmp,
                                    axis=mybir.AxisListType.X,
                                    op=ALU.add,
                                )
                        if it == 1:
                            # capsule_weight accumulates: cw1 += cw0
                            nc.vector.tensor_add(cw1, cw1, cw0)

                # write this phase's capsule out
                nc.vector.tensor_copy(out=capx[:, 0, :], in_=cap)
                nc.sync.dma_start(
                    out=out_d[:, jlo : jlo + H], in_=capx[:, 0, :]
                )

    _legalize_waits(nc)
    return nc


def kernel(item_eb: np.ndarray, mask: np.ndarray, w: np.ndarray) -> np.ndarray:
    item_eb = np.asarray(item_eb, dtype=np.float32)
    mask_i = np.asarray(mask)
    w = np.asarray(w, dtype=np.float32)

    # host-side layout prep (no math)
    xt = np.ascontiguousarray(item_eb.transpose(1, 2, 0))  # [S, H, B] f32
    wt = np.ascontiguousarray(w[0].transpose(0, 2, 1))  # [S, H, IH] f32
    mskf = np.ascontiguousarray(mask_i.astype(np.float32))  # [B, S]

    if "nc" not in _cache:
        _cache["nc"] = _build_program()
    nc = _cache["nc"]

    in_maps = []
    for c in range(NCORES):
        b0, b1 = c * BC, (c + 1) * BC
        in_maps.append(
            {
                "xt": np.ascontiguousarray(xt[:, :, b0:b1]),
                "wt": wt,
                "msk": np.ascontiguousarray(mskf[b0:b1, :]),
            }
        )

    outs = [None] * NCORES
    try:
        res = run_bass_kernel_spmd(nc, in_maps, list(range(NCORES)))
        for c in range(NCORES):
            outs[c] = np.asarray(res.results[c]["out"])
    except Exception:
        # 8-core single dispatch is occasionally flaky through the axon
        # tunnel; fall back to two 4-core groups (still data parallel).
        for grp in ([0, 1, 2, 3], [4, 5, 6, 7]):
            res = run_bass_kernel_spmd(nc, [in_maps[c] for c in grp], grp)
            for i, c in enumerate(grp):
                outs[c] = np.asarray(res.results[i]["out"])
    full = np.concatenate(outs, axis=0).astype(np.float32)  # [B, IH]
    return full.reshape(B, I, H)


if __name__ == "__main__":
    rng = np.random.default_rng(0)
    x = rng.standard_normal((B, S, H), dtype=np.float32)
    m = rng.integers(0, 2, size=(B, S)).astype(np.int32)
    ww = rng.standard_normal((1, S, IH, H), dtype=np.float32)
    o = kernel(item_eb=x, mask=m, w=ww)
    print(o.shape, o.dtype, np.abs(o).mean())

